# revision 1
# baseline (speedup 1.0000x reference)
#!/usr/bin/env python3
"""Multi-head attention (B=16, N=1024, E=768, H=8, softmax-then-scale variant)
as a Bass/Tile kernel on 8 TRN2 NeuronCores, data-parallel over the batch.

Per core (2 batch elements, T=2048 tokens), all matmuls in fp32r (full-rate
PE with ~2^-15 mantissa rounding; measured matmul relerr 3e-5 vs fp32):
  - x fed pre-transposed from host as xT [E, T]; activation/weight DRAM
    tensors are declared float32r so DMA loads them directly (the PE
    truncates the mantissa on read - verified equivalent on HW).
  - loop over batch b, then head h:
      Q^T/K^T: lhsT = Wq[:,h] slice [128,96], rhs = xT chunk -> [96, 1024]
      energy^T per ktile: lhsT = K^T slice [96,128], rhs = Q^T [96,512]
      exp on ScalarE (no max subtraction: |energy| <~ 60 fits fp32 exp)
      attn@V flash-style: lhsT = Vhat [128, 97] (V cols for head h + a
        sqrt(E) constant column so row 96 accumulates sqrt(E)*sumexp),
        rhs = expT [128,512], accumulated over 8 k-tiles -> zT [97, 1024]
      normalize: recip = 1/zT[96] (DVE), replicated across partitions by
        the gpsimd partition_broadcast custom op, z_h = zT[0:96] * recip
    then output projection for batch b: R = sum_h z_h^T.T @ Wo_h + 1^T bo
"""
import os
import sys

sys.path.insert(0, "/opt/trn_rl_repo")

import numpy as np

B, N, E, H, D = 16, 1024, 768, 8, 96
NCORES = 8
BPC = B // NCORES          # batch elements per core
T = BPC * N                # tokens per core
KT = E // 128              # k-tiles over embedding dim (6)
MT = T // 128              # token tiles per core (16)
NKT = N // 128             # k-tiles over sequence (8)

_CACHE = {}


def _build(with_bias=True):
    import concourse.tile as tile
    from concourse import bacc, mybir

    f32 = mybir.dt.float32
    f32r = mybir.dt.float32r

    nc = bacc.Bacc("TRN2", target_bir_lowering=False, debug=False)

    # activation/weight inputs are declared float32r: the PE truncates the
    # mantissa on read, so feeding raw fp32 bits through DMA is equivalent
    # to an on-chip rounding pass (verified on HW)
    xT_d = nc.dram_tensor("xT", [E, T], f32r, kind="ExternalInput").ap()
    wq_d = nc.dram_tensor("wqh", [H, 128, KT, D], f32r, kind="ExternalInput").ap()
    wk_d = nc.dram_tensor("wkh", [H, 128, KT, D], f32r, kind="ExternalInput").ap()
    wv_d = nc.dram_tensor("wv", [E, E], f32r, kind="ExternalInput").ap()
    wo_d = nc.dram_tensor("wo", [E, E], f32r, kind="ExternalInput").ap()
    bqk_d = nc.dram_tensor("bqk", [D, 2 * H], f32, kind="ExternalInput").ap()
    bv_d = nc.dram_tensor("bv1", [1, E], f32r, kind="ExternalInput").ap()
    bo_d = nc.dram_tensor("bo1", [1, E], f32r, kind="ExternalInput").ap()
    out_d = nc.dram_tensor("out", [T, E], f32, kind="ExternalOutput").ap()

    with tile.TileContext(nc) as tc:
        _body(nc, tc, mybir,
              xT_d, wq_d, wk_d, wv_d, wo_d, bqk_d, bv_d, bo_d, out_d,
              with_bias)

    nc.compile()
    return nc


def _body(nc, tc, mybir,
          xT_d, wq_d, wk_d, wv_d, wo_d, bqk_d, bv_d, bo_d, out_d,
          with_bias):
    from contextlib import ExitStack
    from concourse import library_config
    from concourse.tile import add_dep_helper

    f32 = mybir.dt.float32
    f32r = mybir.dt.float32r
    Exp = mybir.ActivationFunctionType.Exp
    ADD = mybir.AluOpType.add
    SQRT_E = float(np.float32(np.sqrt(E)))

    ctx = ExitStack()
    with ctx:
        persist = ctx.enter_context(tc.tile_pool(name="persist", bufs=1))
        qkpool = ctx.enter_context(tc.tile_pool(name="qkpool", bufs=1))
        wqkpool = ctx.enter_context(tc.tile_pool(name="wqkpool", bufs=1))
        projp = ctx.enter_context(tc.tile_pool(name="projp", bufs=2, space="PSUM"))
        dramp = ctx.enter_context(tc.tile_pool(name="dramp", bufs=2, space="DRAM"))
        epp = ctx.enter_context(tc.tile_pool(name="epp", bufs=2, space="PSUM"))
        zp = ctx.enter_context(tc.tile_pool(name="zp", bufs=2, space="PSUM"))

        xt = []
        vhat = []
        wo8 = []
        state = {}

        # ---------------- helpers ----------------
        def proj_head(b, h):
            """Load Wq/Wk slices for head h, compute Q^T/K^T for batch b."""
            tok0 = b * N
            wqr = {}
            for nm, wd in (("q", wq_d), ("k", wk_d)):
                wr = wqkpool.tile([128, KT, D], f32r, name=f"w{nm}r",
                                  tag=f"w{nm}r", bufs=2)
                nc.gpsimd.dma_start(out=wr, in_=wd[h])
                wqr[nm] = wr

            qk = {}
            for i, nm in enumerate(("q", "k")):
                qt = qkpool.tile([D, N], f32r, name=f"{nm}t", tag=f"{nm}t",
                                 bufs=2)
                for tc2 in range(N // 512):
                    pq = projp.tile([128, 512], f32, name="pp", tag="pp")
                    for c in range(KT):
                        nc.tensor.matmul(
                            pq[0:D, :],
                            wqr[nm][:, c, :],
                            xt[c][:, tok0 + tc2 * 512:tok0 + (tc2 + 1) * 512],
                            start=(c == 0), stop=(c == KT - 1),
                        )
                    if with_bias:
                        cp = nc.vector.tensor_scalar(
                            out=qt[:, tc2 * 512:(tc2 + 1) * 512],
                            in0=pq[0:D, :],
                            scalar1=state["bqk_t"][:, i * H + h:i * H + h + 1],
                            scalar2=None, op0=ADD,
                        )
                    else:
                        cp = nc.vector.tensor_copy(
                            out=qt[:, tc2 * 512:(tc2 + 1) * 512],
                            in_=pq[0:D, :],
                        )
                    qk["last_cp"] = cp
                qk[nm] = qt
            return qk

        def attention(b, h, qk):
            """energy -> exp -> attn@V -> normalized z for (b, h)."""
            zT = zp.tile([128, N], f32, name="zT", tag="zT")
            for kt in range(NKT):
                ext = expp.tile([128, N], f32r, name="ext", tag="ext")
                for qc in range(2):
                    ep = epp.tile([128, 512], f32, name="ep", tag="ep")
                    nc.tensor.matmul(
                        ep,
                        qk["k"][:, kt * 128:(kt + 1) * 128],
                        qk["q"][:, qc * 512:(qc + 1) * 512],
                        start=True, stop=True,
                    )
                    nc.scalar.activation(
                        out=ext[:, qc * 512:(qc + 1) * 512], in_=ep, func=Exp)
                    nc.tensor.matmul(
                        zT[0:D + 1, qc * 512:(qc + 1) * 512],
                        vhat[b * NKT + kt][:, h, :],
                        ext[:, qc * 512:(qc + 1) * 512],
                        start=(kt == 0), stop=(kt == NKT - 1),
                    )

            # normalize: z = zT[0:D] / zT[D]  (row D = sqrt(E)*sumexp),
            # split per 512-column half; the recip row is replicated across
            # partitions with the gpsimd partition_broadcast custom
            # instruction (SBUF->SBUF, no DRAM round-trip)
            zth = ztpool.tile([D, N], f32r, name=f"zt{h}", tag=f"zt{h}")
            for qc in range(2):
                sl = slice(qc * 512, (qc + 1) * 512)
                recip = rbp.tile([1, 512], f32, name="recip", tag="recip",
                                 bufs=2)
                nc.vector.reciprocal(out=recip, in_=zT[D:D + 1, sl])
                rb = rbp.tile([D, 512], f32, name="rb", tag="rb")
                nc.gpsimd.partition_broadcast(out_ap=rb, in_ap=recip)
                nc.vector.tensor_mul(out=zth[:, sl], in0=zT[0:D, sl], in1=rb)
            return zth

        def final_proj(b, zt8):
            """Output projection, software-pipelined across 5 psum groups.

            Heads 0..6 of up to 5 (mt, half) groups are accumulated before
            the first h7 matmul, so the PE has ~5us of work while the last
            head's normalize chain (recip -> DRAM round-trip -> mul) is
            still producing zt8[7]. Slots are borrowed from the idle
            energy (ep) and attention-accumulator (zT) pools.
            """
            tok0 = b * N
            groups = [(mt, half) for mt in range(NKT) for half in range(2)]
            DEPTH = 5
            prs = {}
            ros = {}

            def open_group(g):
                mt, half = groups[g]
                k = g % DEPTH
                if k < 2:
                    pr = projp.tile([128, 384], f32, name="pp", tag="pp")
                elif k < 4:
                    pr = epp.tile([128, 384], f32, name="fep", tag="ep")
                else:
                    pr = zp.tile([128, 384], f32, name="fzt", tag="zT")
                cols = slice(half * 384, (half + 1) * 384)
                for h in range(H - 1):
                    nc.tensor.matmul(
                        pr, zt8[h][:, mt * 128:(mt + 1) * 128], wo8[h][:, cols],
                        start=(h == 0), stop=False,
                    )
                prs[g] = pr

            for g in range(min(DEPTH, len(groups))):
                open_group(g)
            for g, (mt, half) in enumerate(groups):
                pr = prs.pop(g)
                cols = slice(half * 384, (half + 1) * 384)
                nc.tensor.matmul(
                    pr, zt8[H - 1][:, mt * 128:(mt + 1) * 128],
                    wo8[H - 1][:, cols],
                    start=False, stop=(not with_bias),
                )
                if with_bias:
                    nc.tensor.matmul(
                        pr, onescol_r, state["bor"][:, cols],
                        start=False, stop=True,
                    )
                if half == 0:
                    ros[mt] = rop.tile([128, E], f32, name="ro", tag="ro")
                if g % 2 == 0:
                    nc.scalar.copy(out=ros[mt][:, cols], in_=pr)
                else:
                    nc.vector.tensor_copy(out=ros[mt][:, cols], in_=pr)
                if g + DEPTH < len(groups):
                    open_group(g + DEPTH)
                # ship each half as soon as its copy lands
                nc.sync.dma_start(
                    out=out_d[tok0 + mt * 128:tok0 + (mt + 1) * 128, cols],
                    in_=ros[mt][:, cols])
                if half == 1:
                    ros.pop(mt)

        # ---------------- phase 0: loads + Vhat ----------------
        qk00 = None
        with tc.tile_pool(name="wvpool", bufs=1) as wvpool:
            for c in range(KT):
                xtc = persist.tile([128, T], f32r, name=f"xt{c}", tag=f"xt{c}")
                xt.append(xtc)

            def load_x_quarter(q):
                for hf in range(2):
                    sl = slice(q * 512 + hf * 256, q * 512 + (hf + 1) * 256)
                    for c in range(KT):
                        nc.sync.dma_start(
                            out=xt[c][:, sl],
                            in_=xT_d[c * 128:(c + 1) * 128, sl])

            # constants
            ones_f = persist.tile([1, 128], f32, name="ones_f", tag="ones_f")
            nc.vector.memset(ones_f, 1.0)
            onescol_r = persist.tile([1, 128], f32r, name="ones_r", tag="ones_r")
            nc.vector.tensor_copy(out=onescol_r, in_=ones_f)
            c27f = persist.tile([128, 1], f32, name="c27f", tag="c27f")
            nc.vector.memset(c27f, SQRT_E)
            c27r = persist.tile([128, 1], f32r, name="c27r", tag="c27r")
            nc.vector.tensor_copy(out=c27r, in_=c27f)

            # first x quarter interleaved with Wv so the Vhat(0) psum
            # group can start accumulating after the first (x, wv) pair;
            # loaded in 256-column halves so Vhat(mt0/mt1) unblock early
            wv = []
            for c in range(KT):
                nc.sync.dma_start(
                    out=xt[c][:, 0:256], in_=xT_d[c * 128:(c + 1) * 128, 0:256])
                wvc = wvpool.tile([128, E], f32r, name=f"wv{c}", tag=f"wv{c}")
                nc.gpsimd.dma_start(out=wvc, in_=wv_d[c * 128:(c + 1) * 128, :])
                wv.append(wvc)
            for c in range(KT):
                nc.sync.dma_start(
                    out=xt[c][:, 256:512],
                    in_=xT_d[c * 128:(c + 1) * 128, 256:512])

            # gpsimd ucode library with partition_broadcast (needed by the
            # first normalize ~35us in; emitted after the Wv loads so it
            # does not head-of-line block the gpsimd DMA queue at startup)
            nc.gpsimd.load_library(library_config.attn)

            # biases
            bqk_t = persist.tile([D, 2 * H], f32, name="bqk_t", tag="bqk_t")
            nc.gpsimd.dma_start(out=bqk_t, in_=bqk_d)
            state["bqk_t"] = bqk_t
            bvr = persist.tile([1, E], f32r, name="bvr", tag="bvr")
            nc.gpsimd.dma_start(out=bvr, in_=bv_d)

            def build_vhat(mt):
                # Vhat[mt] : [128 tokens, H, D+1]; column D holds sqrt(E)
                vh = persist.tile([128, H, D + 1], f32r, name=f"vhat{mt}",
                                  tag=f"vhat{mt}")
                for half in range(2):  # heads 0-3 / 4-7 (384 cols each)
                    pv = projp.tile([128, 512], f32, name="pp", tag="pp")
                    cols = slice(half * 4 * D, (half + 1) * 4 * D)
                    for c in range(KT):
                        nc.tensor.matmul(
                            pv[:, 0:4 * D],
                            xt[c][:, mt * 128:(mt + 1) * 128],
                            wv[c][:, cols],
                            start=(c == 0),
                            stop=(not with_bias and c == KT - 1),
                        )
                    if with_bias:
                        nc.tensor.matmul(
                            pv[:, 0:4 * D], onescol_r, bvr[:, cols],
                            start=False, stop=True,
                        )
                    nc.scalar.copy(
                        out=vh[:, half * 4:(half + 1) * 4, 0:D],
                        in_=pv[:, 0:4 * D].rearrange("p (h d) -> p h d", h=4),
                    )
                nc.vector.tensor_copy(
                    out=vh[:, :, D:D + 1],
                    in_=c27r.to_broadcast([128, H, 1]),
                )
                vhat.append(vh)

            # interleave: quarters 0-1 -> Vhat 0-7, then the first head
            # projection (keeps the PE busy while quarters 2-3 stream in)
            for q in range(2):
                if q > 0:
                    load_x_quarter(q)
                for mt in range(4 * q, 4 * q + 4):
                    build_vhat(mt)
            qk00 = proj_head(0, 0)
            for q in range(2, 4):
                load_x_quarter(q)
                for mt in range(4 * q, 4 * q + 4):
                    build_vhat(mt)

        # stage + wv pools released; later pools reuse their space
        expp = ctx.enter_context(tc.tile_pool(name="expp", bufs=3))
        rbp = ctx.enter_context(tc.tile_pool(name="rbp", bufs=2))
        rop = ctx.enter_context(tc.tile_pool(name="rop", bufs=2))
        ztpool = ctx.enter_context(tc.tile_pool(name="ztpool", bufs=1))
        wopool = ctx.enter_context(tc.tile_pool(name="wopool", bufs=1))

        # Wo -> fp32r per-head tiles + bo (phase 2 operands)
        for h in range(H):
            woh = wopool.tile([D, E], f32r, name=f"wo{h}", tag=f"wo{h}")
            nc.gpsimd.dma_start(out=woh, in_=wo_d[h * D:(h + 1) * D, :])
            wo8.append(woh)
        if with_bias:
            bor = wopool.tile([1, E], f32r, name="bor", tag="bor")
            nc.gpsimd.dma_start(out=bor, in_=bo_d)
            state["bor"] = bor

        # ---------------- phases 1+2, batch-major, software-pipelined ------
        qk_next = qk00
        for b in range(BPC):
            zt8 = []
            for h in range(H):
                qk = qk_next if (h == 0 and qk_next is not None) \
                    else proj_head(b, h)
                qk_next = None
                zt8.append(attention(b, h, qk))
            if b + 1 < BPC:
                # emit next batch's first projection before the output
                # projection so the PE has work while zt(h=7) normalizes
                qk_next = proj_head(b + 1, 0)
            final_proj(b, zt8)


def _get_runner(with_bias=False):
    """Build (once per variant) a jitted shard_map executing the NEFF."""
    key = ("runner", with_bias)
    if key in _CACHE:
        return _CACHE[key]

    import jax
    from jax.experimental.shard_map import shard_map
    from jax.sharding import Mesh, NamedSharding, PartitionSpec
    from concourse import mybir
    from concourse.bass2jax import (
        _bass_exec_p, install_neuronx_cc_hook, partition_id_tensor)

    nc = _build(with_bias=with_bias)
    install_neuronx_cc_hook()

    partition_name = (
        nc.partition_id_tensor.name if nc.partition_id_tensor else None)
    in_names, out_names, out_avals, zero_outs = [], [], [], []
    for alloc in nc.m.functions[0].allocations:
        if not isinstance(alloc, mybir.MemoryLocationSet):
            continue
        name = alloc.memorylocations[0].name
        if alloc.kind == "ExternalInput":
            if name != partition_name:
                in_names.append(name)
        elif alloc.kind == "ExternalOutput":
            out_names.append(name)
            shape = tuple(alloc.tensor_shape)
            dtype = mybir.dt.np(alloc.dtype)
            out_avals.append(jax.core.ShapedArray(shape, dtype))
            zero_outs.append(np.zeros(shape, dtype))
    n_params = len(in_names)
    all_in_names = in_names + out_names
    if partition_name is not None:
        all_in_names = all_in_names + [partition_name]

    def _bass_body(*args):
        operands = list(args)
        if partition_name is not None:
            operands.append(partition_id_tensor())
        outs = _bass_exec_p.bind(
            *operands,
            out_avals=tuple(out_avals),
            in_names=tuple(all_in_names),
            out_names=tuple(out_names),
            lowering_input_output_aliases=(),
            sim_require_finite=True,
            sim_require_nnan=True,
            nc=nc,
        )
        return tuple(outs)

    devices = jax.devices()[:NCORES]
    mesh = Mesh(np.asarray(devices), ("core",))
    spec = PartitionSpec("core")
    rspec = PartitionSpec()          # replicated (weights/biases)
    sharding = NamedSharding(mesh, spec)
    rsharding = NamedSharding(mesh, rspec)
    n_outs = len(out_names)
    # xT is per-core data; everything else is identical across cores
    in_specs = tuple(spec if nm == "xT" else rspec for nm in in_names)
    jitted = jax.jit(
        shard_map(
            _bass_body, mesh=mesh,
            in_specs=in_specs + (spec,) * n_outs,
            out_specs=(spec,) * n_outs,
            check_rep=False,
        ),
        keep_unused=True,
    )
    zeros_dev = [
        jax.device_put(np.concatenate([z] * NCORES, axis=0), sharding)
        for z in zero_outs
    ]
    runner = {
        "jitted": jitted, "in_names": in_names, "out_names": out_names,
        "sharding": sharding, "rsharding": rsharding,
        "zeros_dev": zeros_dev, "jax": jax,
    }
    _CACHE[key] = runner
    return runner


def _prep_inputs(x, Wq, bq, Wk, bk, Wv, bv, Wo, bo):
    """Host-side prep: arrays keyed by NEFF input name. xT is per-core
    concatenated; weights/biases are single copies (replicated spec)."""
    x = np.asarray(x, dtype=np.float32)
    Wq, Wk, Wv, Wo = (np.asarray(w, dtype=np.float32) for w in (Wq, Wk, Wv, Wo))
    bq, bk, bv, bo = (np.asarray(v, dtype=np.float32) for v in (bq, bk, bv, bo))

    xcat = np.ascontiguousarray(
        x.reshape(NCORES, T, E).transpose(0, 2, 1)).reshape(NCORES * E, T)
    # [H, 128, KT, D]: per-head slices DMA with 2304B-contiguous rows
    wqh = np.ascontiguousarray(
        Wq.reshape(KT, 128, H, D).transpose(2, 1, 0, 3))
    wkh = np.ascontiguousarray(
        Wk.reshape(KT, 128, H, D).transpose(2, 1, 0, 3))
    bqk = np.ascontiguousarray(
        np.concatenate([bq.reshape(H, D).T, bk.reshape(H, D).T], axis=1))

    return {
        "xT": xcat,
        "wqh": wqh, "wkh": wkh, "wv": Wv, "wo": Wo,
        "bqk": bqk, "bv1": np.ascontiguousarray(bv.reshape(1, E)),
        "bo1": np.ascontiguousarray(bo.reshape(1, E)),
    }


def _run(inputs, device_resident=None, with_bias=False):
    r = _get_runner(with_bias)
    args = []
    for name in r["in_names"]:
        if device_resident is not None and name in device_resident:
            args.append(device_resident[name])
        else:
            args.append(inputs[name])
    outs = r["jitted"](*args, *r["zeros_dev"])
    return {name: outs[i] for i, name in enumerate(r["out_names"])}


def _weights_on_device(inputs, with_bias=False):
    """device_put the (replicated) weight/bias arrays once per unique value."""
    import hashlib
    r = _get_runner(with_bias)
    key = hashlib.sha1()
    for name in sorted(inputs):
        if name == "xT":
            continue
        a = inputs[name]
        key.update(name.encode())
        key.update(a.shape.__repr__().encode())
        key.update(a.tobytes())
    key = key.hexdigest()
    cached = _CACHE.get("weights_dev")
    if cached is not None and cached[0] == key:
        return cached[1]
    dev = {
        name: r["jax"].device_put(a, r["rsharding"])
        for name, a in inputs.items() if name != "xT"
    }
    _CACHE["weights_dev"] = (key, dev)
    return dev


def kernel(x, Wq, bq, Wk, bk, Wv, bv, Wo, bo):
    with_bias = any(
        np.any(np.asarray(v)) for v in (bq, bk, bv, bo))
    inputs = _prep_inputs(x, Wq, bq, Wk, bk, Wv, bv, Wo, bo)
    dev = _weights_on_device(inputs, with_bias)
    outs = _run(inputs, dev, with_bias)
    out = np.asarray(outs["out"])          # [NCORES*T, E]
    return out.reshape(B, N, E)


def bench(x, Wq, bq, Wk, bk, Wv, bv, Wo, bo, iters=20):
    """Time repeated executions with all inputs device-resident.

    Returns (per_call_seconds, overhead_floor_seconds)."""
    import time
    r = _get_runner()
    inputs = _prep_inputs(x, Wq, bq, Wk, bk, Wv, bv, Wo, bo)
    dev = _weights_on_device(inputs)
    dev = dict(dev)
    dev["xT"] = r["jax"].device_put(inputs["xT"], r["sharding"])

    out = _run(inputs, dev)
    list(out.values())[0].block_until_ready()

    t0 = time.time()
    last = None
    for _ in range(iters):
        last = _run(inputs, dev)
    for v in last.values():
        v.block_until_ready()
    dt = (time.time() - t0) / iters
    return dt



# revision 71
# speedup vs baseline: 1.1123x; 1.1123x over previous
#!/usr/bin/env python3
"""Multi-head attention (B=16, N=1024, E=768, H=8, softmax-then-scale variant)
as a Bass/Tile kernel on 8 TRN2 NeuronCores, data-parallel over the batch.

v2: full-density projections + mixed precision.
  - Q/K projections contract with [128,128] weight tiles (full PE rows) into
    dense Q^T/K^T [768, N] PSUM tiles; idle engines (DVE/Pool/ACT) repartition
    the 128-row tiles into head-aligned [96, N] SBUF tiles for the energy
    matmuls (features f = 96h+d cross the 128-partition tile boundary).
  - The attn@V output zT_h [97, N] (row 96 = sqrt(E)*sumexp via a constant
    column in Vhat) is normalized and packed head-major into dense
    zfull^T [768, N] bf16 tiles, so the output projection contracts all 768
    rows at full PE density (6x[128,128] lhsT tiles) instead of 8 per-head
    96-row groups.
  - bf16 for V/exp/z/Wo (measured end-to-end ~4e-3 rel err, gate is 2e-2);
    q/k stay f32r (bf16 there costs 1.6e-2 - too close to the gate).
  - Two-batch software pipeline: QKproj(b1) fills the exp-latency stalls in
    batch 0's attention; OutProj(b0) fills batch 1's attention.
"""
import os
import sys

sys.path.insert(0, "/opt/trn_rl_repo")

import numpy as np

B, N, E, H, D = 16, 1024, 768, 8, 96
NCORES = 8
BPC = B // NCORES          # batch elements per core
T = BPC * N                # tokens per core
KT = E // 128              # k-tiles over embedding dim (6)
NKT = N // 128             # k-tiles over sequence (8)

_CACHE = {}


def _pc_segments(pc):
    """PSUM rows [r0,r1) of projection tile pc -> (head h, head-row d0)."""
    segs = []
    r = 0
    while r < 128:
        f = pc * 128 + r
        h, d = f // D, f % D
        run = min(128 - r, D - d)
        segs.append((r, r + run, h, d))
        r += run
    return segs


def _win(p):
    """Max partition span for an engine access starting at partition p:
    windows must not cross their alignment boundary (start 0: 128, start
    64: 64, start 32/96: 32)."""
    if p == 0:
        return 128
    if p % 64 == 0:
        return 64
    return 32


def _zf_segments(h):
    """Head-h z rows [d0,d0+run) -> (zfull tile kc, tile-row t0), split so
    both source (d0) and destination (t0) partition windows are legal."""
    segs = []
    d = 0
    while d < D:
        g = D * h + d
        kc, t0 = g // 128, g % 128
        run = min(128 - t0, D - d, _win(t0), _win(d % 128))
        segs.append((kc, t0, d, run))
        d += run
    return segs


def _build(with_bias=True):
    import concourse.tile as tile
    from concourse import bacc, mybir

    f32 = mybir.dt.float32
    f32r = mybir.dt.float32r
    bf16 = mybir.dt.bfloat16

    nc = bacc.Bacc("TRN2", target_bir_lowering=False, debug=False)

    # activation/weight f32r inputs are fed raw fp32 bits through DMA: the PE
    # truncates the mantissa on read (verified equivalent on HW)
    xT_d = nc.dram_tensor("xT", [E, T], f32r, kind="ExternalInput").ap()
    wqs_d = nc.dram_tensor("wqs", [KT, 128, KT, 128], f32r,
                           kind="ExternalInput").ap()
    wks_d = nc.dram_tensor("wks", [KT, 128, KT, 128], f32r,
                           kind="ExternalInput").ap()
    wv_d = nc.dram_tensor("wv", [E, E], f32r, kind="ExternalInput").ap()
    wo_d = nc.dram_tensor("wo16", [E, E], bf16, kind="ExternalInput").ap()
    bqk_d = nc.dram_tensor("bqk_al", [128, 2 * KT], f32,
                           kind="ExternalInput").ap()
    bv_d = nc.dram_tensor("bv1", [1, E], f32r, kind="ExternalInput").ap()
    bo_d = nc.dram_tensor("bo16", [1, E], bf16, kind="ExternalInput").ap()
    out_d = nc.dram_tensor("out", [T, E], f32, kind="ExternalOutput").ap()

    with tile.TileContext(nc) as tc:
        _body(nc, tc, mybir,
              xT_d, wqs_d, wks_d, wv_d, wo_d, bqk_d, bv_d, bo_d, out_d,
              with_bias)

    nc.compile()
    return nc


def _body(nc, tc, mybir,
          xT_d, wqs_d, wks_d, wv_d, wo_d, bqk_d, bv_d, bo_d, out_d,
          with_bias):
    from collections import deque
    from contextlib import ExitStack
    from concourse import library_config

    f32 = mybir.dt.float32
    f32r = mybir.dt.float32r
    bf16 = mybir.dt.bfloat16
    Exp = mybir.ActivationFunctionType.Exp
    ADD = mybir.AluOpType.add
    SQRT_E = float(np.float32(np.sqrt(E)))

    ctx = ExitStack()
    with ctx:
        persist = ctx.enter_context(tc.tile_pool(name="persist", bufs=1))
        reshp = ctx.enter_context(tc.tile_pool(name="reshp", bufs=2))
        wqkp = ctx.enter_context(tc.tile_pool(name="wqkp", bufs=2))
        qkp = ctx.enter_context(tc.tile_pool(name="qkp", bufs=1))
        vhp = ctx.enter_context(tc.tile_pool(name="vhp", bufs=1))
        pp = ctx.enter_context(tc.tile_pool(name="pp", bufs=2, space="PSUM"))
        epp = ctx.enter_context(tc.tile_pool(name="epp", bufs=2, space="PSUM"))
        zp = ctx.enter_context(tc.tile_pool(name="zp", bufs=2, space="PSUM"))

        xt = []          # 6 x [128, T] f32r
        wv = []          # 6 x [128, E] f32r
        wo = []          # 6 x [128, E] bf16
        vhat = {}        # mt -> [128 tok, H, D+1] bf16
        qh = {}          # (b,h) -> [D, N] f32r   (current batch only)
        kh = {}
        state = {}

        # ---------------- helpers ----------------
        def load_strip(mat, wd, pc):
            wr = wqkp.tile([128, KT, 128], f32r, name=f"w{mat}{pc}",
                           tag=f"w{mat}")
            nc.gpsimd.dma_start(out=wr, in_=wd[pc])
            return wr

        # copy-engine rotation for repartition / output copies. All these
        # copies read PSUM, which GPSIMD cannot access on real HW, so only
        # DVE (vector) and ACT (scalar.copy) are eligible; ACT only when it
        # is not saturated with exp.
        def cp_copy(i, allow_act, out, in_):
            if allow_act and i % 2:
                nc.scalar.copy(out=out, in_=in_)
            else:
                nc.vector.tensor_copy(out=out, in_=in_)

        def qkproj_unit(b, pc, mi, ch, strips, allow_act, ci=0,
                        dma_eng=None):
            """One Q^T/K^T projection group: rows [pc*128,(pc+1)*128),
            token chunk ch, matrix mi (0=q, 1=k), for batch b.

            Engines cannot access partition windows that cross their
            alignment boundary (e.g. 64 partitions starting at 32), so the
            head repartitioning is done by SBUF->SBUF DMAs (no partition
            window limits) from a full-window scratch copy."""
            tok0 = b * N
            mat = "qk"[mi]
            dst = qh if mi == 0 else kh
            pq = pp.tile([128, 512], f32, name="pp", tag="pp")
            for kc in range(KT):
                nc.tensor.matmul(
                    pq,
                    strips[mat][:, kc, :],
                    xt[kc][:, tok0 + ch * 512:tok0 + (ch + 1) * 512],
                    start=(kc == 0), stop=(kc == KT - 1),
                )
            sc = reshp.tile([128, 512], f32r, name="sc", tag="sc")
            if with_bias:
                nc.vector.tensor_scalar(
                    out=sc, in0=pq,
                    scalar1=state["bqk"][:, pc * 2 + mi:pc * 2 + mi + 1],
                    scalar2=None, op0=ADD,
                )
            else:
                cp_copy(ci, allow_act, sc, pq)
            sl = slice(ch * 512, (ch + 1) * 512)
            for r0, r1, h, d0 in _pc_segments(pc):
                (dma_eng or nc.gpsimd).dma_start(
                    out=dst[(b, h)][d0:d0 + (r1 - r0), sl],
                    in_=sc[r0:r1, :])

        def qkproj(b, pc, strips, allow_act=False):
            """Phase-0 variant: repartition via direct window-legal engine
            copies (ACT and DVE are both nearly idle before the first
            attention), no scratch/DMA hop."""
            tok0 = b * N
            ci = pc
            for mi in range(2):
                dst = qh if mi == 0 else kh
                for ch in range(2):
                    pq = pp.tile([128, 512], f32, name="pp", tag="pp")
                    for kc in range(KT):
                        nc.tensor.matmul(
                            pq,
                            strips["qk"[mi]][:, kc, :],
                            xt[kc][:, tok0 + ch * 512:tok0 + (ch + 1) * 512],
                            start=(kc == 0), stop=(kc == KT - 1),
                        )
                    sl = slice(ch * 512, (ch + 1) * 512)
                    for r0, r1, h, d0 in _pc_segments(pc):
                        run = 0
                        while run < r1 - r0:
                            piece = min(_win((r0 + run) % 128),
                                        _win((d0 + run) % 128),
                                        r1 - r0 - run)
                            if with_bias:
                                nc.vector.tensor_scalar(
                                    out=dst[(b, h)][d0 + run:
                                                    d0 + run + piece, sl],
                                    in0=pq[r0 + run:r0 + run + piece, :],
                                    scalar1=state["bqk"][
                                        r0 + run:r0 + run + piece,
                                        pc * 2 + mi:pc * 2 + mi + 1],
                                    scalar2=None, op0=ADD,
                                )
                            else:
                                cp_copy(ci, True,
                                        dst[(b, h)][d0 + run:
                                                    d0 + run + piece, sl],
                                        pq[r0 + run:r0 + run + piece, :])
                            ci += 1
                            run += piece

        def new_qk_tiles(b):
            for h in range(H):
                qh[(b, h)] = qkp.tile([D, N], f32r, name=f"q{h}", tag=f"q{h}")
                kh[(b, h)] = qkp.tile([D, N], f32r, name=f"k{h}", tag=f"k{h}")

        def build_vhat(mt, dve_copy=False):
            """Vhat[mt]: [128 tok, H, D+1] bf16; col D holds sqrt(E).
            dve_copy routes the psum copies DVE-only (for units pumped while
            ACT is saturated with exp)."""
            vh = vhp.tile([128, H, D + 1], bf16, name=f"vhat{mt}",
                          tag=f"vhat{mt}")
            for half in range(2):  # heads 0-3 / 4-7 (384 cols each)
                pv = pp.tile([128, 384], f32, name="pp", tag="pp")
                cols = slice(half * 4 * D, (half + 1) * 4 * D)
                for kc in range(KT):
                    nc.tensor.matmul(
                        pv,
                        xt[kc][:, mt * 128:(mt + 1) * 128],
                        wv[kc][:, cols],
                        start=(kc == 0),
                        stop=(not with_bias and kc == KT - 1),
                    )
                if with_bias:
                    nc.tensor.matmul(
                        pv, state["ones_r"], state["bvr"][:, cols],
                        start=False, stop=True,
                    )
                # vector/scalar only: f32->bf16 conversion on gpsimd is
                # not trusted
                if dve_copy or (mt + half) % 2:
                    nc.vector.tensor_copy(
                        out=vh[:, half * 4:(half + 1) * 4, 0:D],
                        in_=pv.rearrange("p (h d) -> p h d", h=4),
                    )
                else:
                    nc.scalar.copy(
                        out=vh[:, half * 4:(half + 1) * 4, 0:D],
                        in_=pv.rearrange("p (h d) -> p h d", h=4),
                    )
            nc.vector.tensor_copy(
                out=vh[:, :, D:D + 1],
                in_=state["c27b"].to_broadcast([128, H, 1]),
            )
            vhat[mt] = vh

        # ---------------- filler queue -------------------------------------
        # Units of independent PE work (~1-2us each) pumped inside the
        # attention inner loop: attention alone needs ~6.8us/head of PE but
        # ~9.8us/head of ACT (exp), so without filler the PE idles ~3us/head.
        filler = deque()

        def pump(n=1):
            k = 0
            while filler and k < n:
                filler.popleft()()
                k += 1

        ros = {}

        def outproj_unit(b, mt, cc, allow_act, ci=0):
            """One output-projection group (token tile mt, column half cc)."""
            tok0 = b * N
            zfs = state[("zf", b)]
            cols = slice(cc * 384, (cc + 1) * 384)
            pr = pp.tile([128, 384], f32, name="pp", tag="pp")
            for kc in range(KT):
                nc.tensor.matmul(
                    pr,
                    zfs[kc][:, mt * 128:(mt + 1) * 128],
                    wo[kc][:, cols],
                    start=(kc == 0),
                    stop=(kc == KT - 1 and not with_bias),
                )
            if with_bias:
                nc.tensor.matmul(
                    pr, state["ones16"], state["bor"][:, cols],
                    start=False, stop=True,
                )
            if cc == 0:
                ros[(b, mt)] = rop.tile([128, E], f32, name="rod", tag="rod",
                                        bufs=2)
            cp_copy(mt + cc + ci, allow_act, ros[(b, mt)][:, cols], pr)
            if cc == 1:
                nc.sync.dma_start(
                    out=out_d[tok0 + mt * 128:tok0 + (mt + 1) * 128, :],
                    in_=ros.pop((b, mt)))

        def enqueue_qkproj(b, pc):
            strips = {"q": load_strip("q", wqs_d, pc),
                      "k": load_strip("k", wks_d, pc)}
            for mi in range(2):
                for ch in range(2):
                    filler.append(
                        lambda mi=mi, ch=ch: qkproj_unit(
                            b, pc, mi, ch, strips, False, ci=pc + mi + ch))

        def enqueue_qkproj2(b, pc, strips):
            for mi in range(2):
                for ch in range(2):
                    filler.append(
                        lambda mi=mi, ch=ch: qkproj_unit(
                            b, pc, mi, ch, strips, False, ci=pc + mi + ch))

        def enqueue_vhat(mts):
            for mt in mts:
                filler.append(lambda mt=mt: build_vhat(mt, dve_copy=True))

        def enqueue_outproj(b, mts):
            for mt in mts:
                for cc in range(2):
                    filler.append(
                        lambda mt=mt, cc=cc: outproj_unit(b, mt, cc, False))

        def attention(b, h, zfs, pump_every=2, drain_queue=False):
            """energy -> exp -> attn@V -> normalize into packed zfull.

            The head is software-pipelined by 2 k-tiles: energies (and their
            exps) run 2 tiles ahead of the attnV consumers, so the ACT
            engine stays saturated and attnV never waits on a fresh exp.
            Requires expp bufs>=3."""
            zT = zp.tile([128, N], f32, name="zT", tag="zT")
            exts = {}

            def attnv(kt):
                for qc in range(2):
                    sl = slice(qc * 512, (qc + 1) * 512)
                    nc.tensor.matmul(
                        zT[0:D + 1, sl],
                        vhat[b * NKT + kt][:, h, :],
                        exts.pop(kt)[:, sl] if qc else exts[kt][:, sl],
                        start=(kt == 0), stop=(kt == NKT - 1),
                    )

            for kt in range(NKT):
                ext = expp.tile([128, N], bf16, name="ext", tag="ext")
                exts[kt] = ext
                for qc in range(2):
                    sl = slice(qc * 512, (qc + 1) * 512)
                    ep = epp.tile([128, 512], f32, name="ep", tag="ep")
                    nc.tensor.matmul(
                        ep,
                        kh[(b, h)][:, kt * 128:(kt + 1) * 128],
                        qh[(b, h)][:, sl],
                        start=True, stop=True,
                    )
                    nc.scalar.activation(out=ext[:, sl], in_=ep, func=Exp)
                if kt >= 2:
                    attnv(kt - 2)
                if kt % pump_every == pump_every - 1:
                    pump(1)
            for kt in range(NKT - 2, NKT):
                pump(1)
                attnv(kt)
            if drain_queue:
                # empty the filler queue BEFORE the normalize emission so
                # the leftover units' copies are not wedged between the
                # normalize muls and the output-projection drain on DVE
                while filler:
                    pump(1)
            # normalize: z = zT[0:D] / zT[D]; recip row replicated across
            # partitions by the gpsimd partition_broadcast custom op.
            # Processed in 512-column halves so each half's
            # recip->broadcast->mul chain runs independently: halves the
            # latency until zfull completes (the last head's chain gates the
            # output-projection drain) and frees zT for the next head sooner.
            w = N // 2
            for qc in range(2):
                sl = slice(qc * w, (qc + 1) * w)
                recip = rbp.tile([1, w], f32, name="recip", tag="recip",
                                 bufs=2)
                nc.vector.reciprocal(out=recip, in_=zT[D:D + 1, sl])
                rb = rbp.tile([D, w], f32, name="rb", tag="rb", bufs=2)
                nc.gpsimd.partition_broadcast(out_ap=rb, in_ap=recip)
                for kc, t0, d0, run in _zf_segments(h):
                    nc.vector.tensor_mul(
                        out=zfs[kc][t0:t0 + run, sl],
                        in0=zT[d0:d0 + run, sl], in1=rb[d0:d0 + run, :])

        def outproj_drain(b):
            """Final output projection, software-pipelined across 4 psum
            groups (pp x2 + borrowed epp x2): the first 4 accumulations of
            zfull tiles 0-4 (heads <=h6) are emitted before the first tile-5
            matmul, so the PE has ~3us of work while h7's normalize chain
            completes. Out-DMAs alternate sync/scalar queues so the tail
            drains twice as fast."""
            tok0 = b * N
            zfs = state[("zf", b)]
            groups = [(mt, cc) for mt in range(NKT) for cc in range(2)]
            DEPTH = 5
            prs = {}

            def open_group(g):
                mt, cc = groups[g]
                k = g % DEPTH
                # slots pp,pp,ep,ep + one zp borrow (h6's old accumulator —
                # h7's own zT slot is never borrowed, it frees too late)
                if k < 2:
                    pr = pp.tile([128, 384], f32, name="pp", tag="pp")
                elif k < 4:
                    pr = epp.tile([128, 384], f32, name="fep", tag="ep")
                else:
                    pr = zp.tile([128, 384], f32, name="fzt", tag="zT")
                cols = slice(cc * 384, (cc + 1) * 384)
                for kc in range(KT - 1):
                    nc.tensor.matmul(
                        pr, zfs[kc][:, mt * 128:(mt + 1) * 128],
                        wo[kc][:, cols], start=(kc == 0), stop=False)
                prs[g] = pr

            for g in range(min(DEPTH, len(groups))):
                open_group(g)
            rod = None
            for g, (mt, cc) in enumerate(groups):
                pr = prs.pop(g)
                cols = slice(cc * 384, (cc + 1) * 384)
                nc.tensor.matmul(
                    pr, zfs[KT - 1][:, mt * 128:(mt + 1) * 128],
                    wo[KT - 1][:, cols],
                    start=False, stop=(not with_bias))
                if with_bias:
                    nc.tensor.matmul(
                        pr, state["ones16"], state["bor"][:, cols],
                        start=False, stop=True,
                    )
                if cc == 0:
                    rod = rop.tile([128, E], f32, name="rod", tag="rod",
                                   bufs=2)
                # halves copied by different engines (DVE / ACT), one DMA
                # per token tile: fewer dispatch chains in the serial tail
                if cc == 0:
                    nc.vector.tensor_copy(out=rod[:, cols], in_=pr)
                else:
                    nc.scalar.copy(out=rod[:, cols], in_=pr)
                if g + DEPTH < len(groups):
                    open_group(g + DEPTH)
                if cc == 1:
                    eng = nc.sync if mt % 2 == 0 else nc.scalar
                    eng.dma_start(
                        out=out_d[tok0 + mt * 128:tok0 + (mt + 1) * 128, :],
                        in_=rod)

        # ---------------- phase 0: loads + QKproj(b0) pc0-2 + Vhat(all) ----
        # Every DMA transfer exclusively holds the shared DMA engines
        # (~360GB/s), so transfers serialize globally: what matters is the
        # ISSUE ORDER, not the queue. All phase-0 loads go on the sync queue
        # in strict priority order matching when the PE needs them.
        with tc.tile_pool(name="wvpool", bufs=1) as wvpool:
            for c in range(KT):
                xtc = persist.tile([128, T], f32r, name=f"xt{c}", tag=f"xt{c}")
                xt.append(xtc)

            def load_x_chunk(ch):
                for c in range(KT):
                    nc.sync.dma_start(
                        out=xt[c][:, ch * 512:(ch + 1) * 512],
                        in_=xT_d[c * 128:(c + 1) * 128,
                                 ch * 512:(ch + 1) * 512])

            def load_strips2(pc):
                wrq = wqkp.tile([128, KT, 128], f32r, name=f"wq{pc}", tag="wq")
                nc.sync.dma_start(out=wrq, in_=wqs_d[pc])
                wrk = wqkp.tile([128, KT, 128], f32r, name=f"wk{pc}", tag="wk")
                nc.sync.dma_start(out=wrk, in_=wks_d[pc])
                return {"q": wrq, "k": wrk}

            # priority order matching the PE order (qkproj pc0..pc4, then
            # vhat 0-15): strips pc0, x chunk0-1, strips pc1-pc4, Wv,
            # x chunk2-3, wo, biases
            strips = load_strips2(0)
            load_x_chunk(0)
            load_x_chunk(1)
            next_strips = load_strips2(1)
            strips_pc2 = load_strips2(2)
            for c in range(KT):
                wvc = wvpool.tile([128, E], f32r, name=f"wv{c}", tag=f"wv{c}")
                nc.sync.dma_start(out=wvc, in_=wv_d[c * 128:(c + 1) * 128, :])
                wv.append(wvc)
            load_x_chunk(2)
            load_x_chunk(3)
            for c in range(KT):
                woc = persist.tile([128, E], bf16, name=f"wo{c}", tag=f"wo{c}")
                nc.sync.dma_start(out=woc, in_=wo_d[c * 128:(c + 1) * 128, :])
                wo.append(woc)
            if with_bias:
                bqk = persist.tile([128, 2 * KT], f32, name="bqk", tag="bqk")
                nc.sync.dma_start(out=bqk, in_=bqk_d)
                state["bqk"] = bqk
                bvr = persist.tile([1, E], f32r, name="bvr", tag="bvr")
                nc.sync.dma_start(out=bvr, in_=bv_d)
                state["bvr"] = bvr
                bor = persist.tile([1, E], bf16, name="bor", tag="bor")
                nc.sync.dma_start(out=bor, in_=bo_d)
                state["bor"] = bor

            # gpsimd ucode library with partition_broadcast (needed by the
            # first normalize, ~60us in)
            nc.gpsimd.load_library(library_config.attn)

            # constants (memset only writes f32; bf16/f32r tiles are
            # filled via converting copies)
            ones_f = persist.tile([1, 128], f32, name="ones_f", tag="ones_f")
            nc.vector.memset(ones_f, 1.0)
            ones_r = persist.tile([1, 128], f32r, name="ones_r", tag="ones_r")
            nc.vector.tensor_copy(out=ones_r, in_=ones_f)
            state["ones_r"] = ones_r
            c27f = persist.tile([128, 1], f32, name="c27f", tag="c27f")
            nc.vector.memset(c27f, SQRT_E)
            c27b = persist.tile([128, 1], bf16, name="c27b", tag="c27b")
            nc.vector.tensor_copy(out=c27b, in_=c27f)
            state["c27b"] = c27b
            if with_bias:
                ones16 = persist.tile([1, 128], bf16, name="ones16",
                                      tag="ones16")
                nc.vector.tensor_copy(out=ones16, in_=ones_f)
                state["ones16"] = ones16

            new_qk_tiles(0)
            qkproj(0, 0, strips, allow_act=True)
            qkproj(0, 1, next_strips, allow_act=True)
            qkproj(0, 2, strips_pc2, allow_act=True)
            for mt in range(16):
                build_vhat(mt)

        # wv pool released; later pools reuse its space
        expp = ctx.enter_context(tc.tile_pool(name="expp", bufs=3))
        rbp = ctx.enter_context(tc.tile_pool(name="rbp", bufs=1))
        rop = ctx.enter_context(tc.tile_pool(name="rop", bufs=2))
        zfp = ctx.enter_context(tc.tile_pool(name="zfp", bufs=2))

        # ---------------- batch 0 attention --------------------------------
        # filler: QKproj(b0) pc3-5 (fresh slots, safe anytime; consumed by
        # b0 heads 4-7) then QKproj(b1). A b1 pc's units can only be
        # enqueued once b0's attention is done with the heads whose q/k tile
        # slots it overwrites (pc0:{0,1} pc1:{1,2} pc2:{2,3} pc3:{4,5}
        # pc4:{5,6} pc5:{6,7}).
        state[("zf", 0)] = [
            zfp.tile([128, N], bf16, name=f"zf{kc}", tag=f"zf{kc}")
            for kc in range(KT)]
        new_qk_tiles(1)
        enqueue_qkproj(0, 3)
        b0_enq = {0: (0, 4), 1: (0, 5), 2: (1, 0), 3: (1, 1), 4: (1, 2),
                  6: (1, 3), 7: (1, 4)}
        for h in range(H):
            attention(0, h, state[("zf", 0)])
            if h in b0_enq:
                enqueue_qkproj(*b0_enq[h])
            pump(1)

        # ---------------- batch 1 attention --------------------------------
        # filler: QKproj(b1) pc5 + OutProj(b0), pumped at a slower cadence
        # (20 units over 8 heads)
        state[("zf", 1)] = [
            zfp.tile([128, N], bf16, name=f"zf{kc}", tag=f"zf{kc}")
            for kc in range(KT)]
        for h in range(H):
            if h == 0:
                enqueue_qkproj(1, 5)    # touches b0 {6,7}: done; b1 h6 needs
            elif h == 1:
                enqueue_outproj(0, range(4))
            elif h == 3:
                enqueue_outproj(0, range(4, 8))
            attention(1, h, state[("zf", 1)], pump_every=6,
                      drain_queue=(h == H - 1))

        # ---------------- drain: OutProj(b1), pipelined --------------------
        outproj_drain(1)


def _get_runner(with_bias=False):
    """Build (once per variant) a jitted shard_map executing the NEFF."""
    key = ("runner", with_bias)
    if key in _CACHE:
        return _CACHE[key]

    import jax
    from jax.experimental.shard_map import shard_map
    from jax.sharding import Mesh, NamedSharding, PartitionSpec
    from concourse import mybir
    from concourse.bass2jax import (
        _bass_exec_p, install_neuronx_cc_hook, partition_id_tensor)

    nc = _build(with_bias=with_bias)
    install_neuronx_cc_hook()

    partition_name = (
        nc.partition_id_tensor.name if nc.partition_id_tensor else None)
    in_names, out_names, out_avals, zero_outs = [], [], [], []
    for alloc in nc.m.functions[0].allocations:
        if not isinstance(alloc, mybir.MemoryLocationSet):
            continue
        name = alloc.memorylocations[0].name
        if alloc.kind == "ExternalInput":
            if name != partition_name:
                in_names.append(name)
        elif alloc.kind == "ExternalOutput":
            out_names.append(name)
            shape = tuple(alloc.tensor_shape)
            dtype = mybir.dt.np(alloc.dtype)
            out_avals.append(jax.core.ShapedArray(shape, dtype))
            zero_outs.append(np.zeros(shape, dtype))
    n_params = len(in_names)
    all_in_names = in_names + out_names
    if partition_name is not None:
        all_in_names = all_in_names + [partition_name]

    def _bass_body(*args):
        operands = list(args)
        if partition_name is not None:
            operands.append(partition_id_tensor())
        outs = _bass_exec_p.bind(
            *operands,
            out_avals=tuple(out_avals),
            in_names=tuple(all_in_names),
            out_names=tuple(out_names),
            lowering_input_output_aliases=(),
            sim_require_finite=True,
            sim_require_nnan=True,
            nc=nc,
        )
        return tuple(outs)

    devices = jax.devices()[:NCORES]
    mesh = Mesh(np.asarray(devices), ("core",))
    spec = PartitionSpec("core")
    rspec = PartitionSpec()          # replicated (weights/biases)
    sharding = NamedSharding(mesh, spec)
    rsharding = NamedSharding(mesh, rspec)
    n_outs = len(out_names)
    # xT is per-core data; everything else is identical across cores
    in_specs = tuple(spec if nm == "xT" else rspec for nm in in_names)
    jitted = jax.jit(
        shard_map(
            _bass_body, mesh=mesh,
            in_specs=in_specs + (spec,) * n_outs,
            out_specs=(spec,) * n_outs,
            check_rep=False,
        ),
        keep_unused=True,
    )
    zeros_dev = [
        jax.device_put(np.concatenate([z] * NCORES, axis=0), sharding)
        for z in zero_outs
    ]
    runner = {
        "jitted": jitted, "in_names": in_names, "out_names": out_names,
        "sharding": sharding, "rsharding": rsharding,
        "zeros_dev": zeros_dev, "jax": jax,
    }
    _CACHE[key] = runner
    return runner


def _prep_inputs(x, Wq, bq, Wk, bk, Wv, bv, Wo, bo):
    """Host-side prep: arrays keyed by NEFF input name. xT is per-core
    concatenated; weights/biases are single copies (replicated spec)."""
    import ml_dtypes
    x = np.asarray(x, dtype=np.float32)
    Wq, Wk, Wv, Wo = (np.asarray(w, dtype=np.float32) for w in (Wq, Wk, Wv, Wo))
    bq, bk, bv, bo = (np.asarray(v, dtype=np.float32) for v in (bq, bk, bv, bo))

    xcat = np.ascontiguousarray(
        x.reshape(NCORES, T, E).transpose(0, 2, 1)).reshape(NCORES * E, T)
    # [pc, kr, kc, pcol]: strip pc is one contiguous 393KB DMA with
    # 3072B-contiguous per-partition rows
    wqs = np.ascontiguousarray(
        Wq.reshape(KT, 128, KT, 128).transpose(2, 1, 0, 3))
    wks = np.ascontiguousarray(
        Wk.reshape(KT, 128, KT, 128).transpose(2, 1, 0, 3))
    # per-partition bias columns aligned to projection row tiles
    bqk_al = np.zeros((128, 2 * KT), dtype=np.float32)
    for pc in range(KT):
        bqk_al[:, pc * 2] = bq[pc * 128:(pc + 1) * 128]
        bqk_al[:, pc * 2 + 1] = bk[pc * 128:(pc + 1) * 128]

    return {
        "xT": xcat,
        "wqs": wqs, "wks": wks, "wv": Wv,
        "wo16": Wo.astype(ml_dtypes.bfloat16),
        "bqk_al": bqk_al,
        "bv1": np.ascontiguousarray(bv.reshape(1, E)),
        "bo16": bo.reshape(1, E).astype(ml_dtypes.bfloat16),
    }


def _run(inputs, device_resident=None, with_bias=False):
    r = _get_runner(with_bias)
    args = []
    for name in r["in_names"]:
        if device_resident is not None and name in device_resident:
            args.append(device_resident[name])
        else:
            args.append(inputs[name])
    outs = r["jitted"](*args, *r["zeros_dev"])
    return {name: outs[i] for i, name in enumerate(r["out_names"])}


def _weights_on_device(inputs, with_bias=False):
    """device_put the (replicated) weight/bias arrays once per unique value."""
    import hashlib
    r = _get_runner(with_bias)
    key = hashlib.sha1()
    for name in sorted(inputs):
        if name == "xT":
            continue
        a = inputs[name]
        key.update(name.encode())
        key.update(a.shape.__repr__().encode())
        key.update(a.tobytes())
    key = key.hexdigest()
    cached = _CACHE.get("weights_dev")
    if cached is not None and cached[0] == key:
        return cached[1]
    dev = {
        name: r["jax"].device_put(a, r["rsharding"])
        for name, a in inputs.items() if name != "xT"
    }
    _CACHE["weights_dev"] = (key, dev)
    return dev


def kernel(x, Wq, bq, Wk, bk, Wv, bv, Wo, bo):
    with_bias = any(
        np.any(np.asarray(v)) for v in (bq, bk, bv, bo))
    inputs = _prep_inputs(x, Wq, bq, Wk, bk, Wv, bv, Wo, bo)
    dev = _weights_on_device(inputs, with_bias)
    outs = _run(inputs, dev, with_bias)
    out = np.asarray(outs["out"])          # [NCORES*T, E]
    return out.reshape(B, N, E)


def bench(x, Wq, bq, Wk, bk, Wv, bv, Wo, bo, iters=20):
    """Time repeated executions with all inputs device-resident.

    Returns per_call_seconds."""
    import time
    r = _get_runner()
    inputs = _prep_inputs(x, Wq, bq, Wk, bk, Wv, bv, Wo, bo)
    dev = _weights_on_device(inputs)
    dev = dict(dev)
    dev["xT"] = r["jax"].device_put(inputs["xT"], r["sharding"])

    out = _run(inputs, dev)
    list(out.values())[0].block_until_ready()

    t0 = time.time()
    last = None
    for _ in range(iters):
        last = _run(inputs, dev)
    for v in last.values():
        v.block_until_ready()
    dt = (time.time() - t0) / iters
    return dt
